# revision 1
# baseline (speedup 1.0000x reference)
import numpy as np
import concourse.bass as bass
import concourse.bacc as bacc
import concourse.mybir as mybir
from concourse.bass_utils import run_bass_kernel_spmd
from concourse import tile

# DigitCapsules dynamic routing, data-parallel over batch on 8 cores.
# B=512, R=1152, C=10, O=16, I=8; per core Bl=64.
#
# Device layout: partitions p = parity*64 + b  (r = 2*rp + parity), so every
# per-(b,r) routing quantity is partition-local. u_hat for a chunk of r-pairs
# is generated by K=16 matmuls whose stationary operand is a block-diagonal
# "pair canvas" [16, 128] staged on-chip from the compact x-pair input.
# Iteration 1 uses uniform coupling, so s_1 = 0.1 * sum_r u_hat is computed
# as a single dense PE accumulation over all (r, i) with no u_hat storage.

NCORES = 8
B, R, C, O, I = 512, 1152, 10, 16, 8
Bl = B // NCORES          # 64 batch per core
CO = C * O                # 160
NP = R // 2               # 576 r-pairs
PAIRS_PER_CHUNK = 24      # 48 r per chunk; 8 psum banks x 3 pairs
NCHUNK = NP // PAIRS_PER_CHUNK  # 24
FCH = PAIRS_PER_CHUNK * CO      # 3840 free elems per chunk
EPS = 1e-8

_cache = {}


def _build_program():
    if "nc" in _cache:
        return _cache["nc"]
    nc = bacc.Bacc("TRN2", target_bir_lowering=False, debug=False)
    f32 = mybir.dt.float32
    xp_d = nc.dram_tensor("xp", [16, NP, 64], f32, kind="ExternalInput")
    wpair_d = nc.dram_tensor("wpair", [16, NP, CO], f32, kind="ExternalInput")
    NG = NP // 8              # 72 stacked groups of 8 pairs (K=128)
    xps_d = nc.dram_tensor("xps", [128, NG, 64], f32, kind="ExternalInput")
    wps_d = nc.dram_tensor("wps", [128, NG, CO], f32, kind="ExternalInput")
    out_d = nc.dram_tensor("v_out", [Bl, CO], f32, kind="ExternalOutput")

    AX = mybir.AxisListType
    ALU = mybir.AluOpType
    ACTF = mybir.ActivationFunctionType

    def ap(t, dims, offset=0):
        return bass.AP(t.tensor, offset, dims)

    with tile.TileContext(nc) as tc:
        with (
            tc.tile_pool(name="xpp", bufs=3) as xp_pool,
            tc.tile_pool(name="stk", bufs=2) as stk_pool,
            tc.tile_pool(name="cv", bufs=2) as cv_pool,
            tc.tile_pool(name="wp", bufs=2) as wp_pool,
            tc.tile_pool(name="psum", bufs=7, space="PSUM") as psum_pool,
            tc.tile_pool(name="ps1", bufs=1, space="PSUM") as ps1_pool,
            tc.tile_pool(name="uch", bufs=2) as uch_pool,
            tc.tile_pool(name="tmp", bufs=1) as tmp_pool,
            tc.tile_pool(name="res", bufs=1) as res_pool,
        ):
            b_ij = res_pool.tile([128, NP * C], f32, tag="bij")       # logits, (rp, c)
            s_acc = res_pool.tile([128, CO], f32, tag="sacc")
            vtile = res_pool.tile([128, CO], f32, tag="vt")           # v on both halves
            s_fold = res_pool.tile([64, CO], f32, tag="sfold")
            sq = res_pool.tile([64, C], f32, tag="sq")
            f1 = res_pool.tile([64, C], f32, tag="f1")
            f2 = res_pool.tile([64, C], f32, tag="f2")

            nc.vector.memset(b_ij[:], 0.0)

            def dma_xp_chunk(k):
                xpch = xp_pool.tile([16, PAIRS_PER_CHUNK * 64], f32, tag="x")
                nc.sync.dma_start(
                    xpch[:],
                    xp_d[:, k * PAIRS_PER_CHUNK:(k + 1) * PAIRS_PER_CHUNK, :],
                )
                return xpch

            def squash_and_store(t):
                # fold parity halves, squash, broadcast v (or emit output)
                upper = tmp_pool.tile([64, CO], f32, tag="up")
                nc.sync.dma_start(upper[:], s_acc[64:128, :])
                nc.vector.tensor_tensor(s_fold[:], s_acc[0:64, :], upper[:], op=ALU.add)
                if t == 0:
                    nc.vector.tensor_scalar_mul(s_fold[:], s_fold[:], 0.1)
                prod = tmp_pool.tile([64, CO], f32, tag="pr")
                nc.vector.tensor_tensor(prod[:], s_fold[:], s_fold[:], op=ALU.mult)
                nc.vector.tensor_reduce(
                    sq[:], ap(prod, [[CO, 64], [16, C], [1, O]]), axis=AX.X, op=ALU.add
                )
                onep = tmp_pool.tile([64, C], f32, tag="q1")
                nc.vector.tensor_scalar_add(onep[:], sq[:], 1.0)
                nc.vector.reciprocal(f1[:], onep[:])
                rt = tmp_pool.tile([64, C], f32, tag="q2")
                nc.vector.tensor_scalar_add(rt[:], sq[:], EPS)
                nc.scalar.activation(rt[:], rt[:], ACTF.Sqrt)
                nc.vector.reciprocal(f2[:], rt[:])
                nc.vector.tensor_tensor(f1[:], f1[:], f2[:], op=ALU.mult)
                nc.vector.tensor_tensor(f1[:], f1[:], sq[:], op=ALU.mult)
                nc.vector.tensor_tensor(
                    vtile[0:64, :], s_fold[:], ap(f1, [[C, 64], [1, C], [0, O]]),
                    op=ALU.mult,
                )
                if t < 2:
                    nc.sync.dma_start(vtile[64:128, :], vtile[0:64, :])
                else:
                    nc.sync.dma_start(out_d[:], vtile[0:64, :])

            # ---- pass A: s_1 = sum_{r,i} x*W, PE only. 8 r-pairs stacked on
            # the contraction (K=128, host-prepped layout) since s_1 sums all r.
            s1ps = ps1_pool.tile([64, CO], f32, tag="s1")
            GA = 9  # stacked groups per chunk (72 total)
            for k in range(NG // GA):
                xk = stk_pool.tile([128, GA * 64], f32, tag="xk")
                nc.sync.dma_start(xk[:], xps_d[:, k * GA:(k + 1) * GA, :])
                wk = stk_pool.tile([128, GA * CO], f32, tag="wk")
                nc.sync.dma_start(wk[:], wps_d[:, k * GA:(k + 1) * GA, :])
                for g in range(GA):
                    nc.tensor.matmul(
                        s1ps[:],
                        xk[:, g * 64:(g + 1) * 64],
                        wk[:, g * CO:(g + 1) * CO],
                        start=(k == 0 and g == 0),
                        stop=(k == NG // GA - 1 and g == GA - 1),
                    )
            nc.vector.memset(s_acc[:], 0.0)
            nc.vector.tensor_copy(s_acc[0:64, :], s1ps[:])
            squash_and_store(0)

            # ---- passes B (t=1), C (t=2): regen u_hat chunks + routing
            cv_count = 0
            for t in (1, 2):
                nc.vector.memset(s_acc[:], 0.0)
                for k in range(NCHUNK):
                    xpch = dma_xp_chunk(k)
                    wch = wp_pool.tile([16, PAIRS_PER_CHUNK * CO], f32, tag="w")
                    nc.sync.dma_start(
                        wch[:],
                        wpair_d[:, k * PAIRS_PER_CHUNK:(k + 1) * PAIRS_PER_CHUNK, :],
                    )
                    # stage block-diagonal canvases: [16, 24*128]
                    cch = cv_pool.tile([16, PAIRS_PER_CHUNK * 128], f32, tag="c")
                    if cv_count < 2:
                        nc.vector.memset(cch[:], 0.0)
                    cv_count += 1
                    # parity 0 rows 0:8 cols rp*128+0:64 ; parity 1 rows 8:16 cols rp*128+64:128
                    cpitch = PAIRS_PER_CHUNK * 128
                    xpitch = PAIRS_PER_CHUNK * 64
                    nc.sync.dma_start(
                        ap(cch, [[cpitch, 8], [128, PAIRS_PER_CHUNK], [1, 64]], 0),
                        ap(xpch, [[xpitch, 8], [64, PAIRS_PER_CHUNK], [1, 64]], 0),
                    )
                    nc.sync.dma_start(
                        ap(cch, [[cpitch, 8], [128, PAIRS_PER_CHUNK], [1, 64]],
                           8 * cpitch + 64),
                        ap(xpch, [[xpitch, 8], [64, PAIRS_PER_CHUNK], [1, 64]],
                           8 * xpitch),
                    )
                    uch = uch_pool.tile([128, FCH], f32, tag="u")
                    for j in range(8):  # 8 psum tiles, 3 pairs each
                        ps = psum_pool.tile([128, 3 * CO], f32, tag="ps")
                        for q in range(3):
                            rp = j * 3 + q
                            nc.tensor.matmul(
                                ps[:, q * CO:(q + 1) * CO],
                                cch[:, rp * 128:(rp + 1) * 128],
                                wch[:, rp * CO:(rp + 1) * CO],
                                start=True, stop=True,
                            )
                        nc.scalar.copy(
                            uch[:, j * 3 * CO:(j + 1) * 3 * CO], ps[:]
                        )

                    # uch free dims: (rp 24, c 10, o 16); strides rp=160, c=16, o=1
                    # a_{t-1}[p,(rp,c)] = sum_o u*v ; b_ij += a
                    tmp = tmp_pool.tile([128, FCH], f32, tag="m1")
                    nc.gpsimd.tensor_tensor(
                        tmp[:], uch[:],
                        ap(vtile, [[CO, 128], [0, PAIRS_PER_CHUNK], [1, CO]]),
                        op=ALU.mult,
                    )
                    ared = tmp_pool.tile([128, PAIRS_PER_CHUNK * C], f32, tag="ar")
                    nc.vector.tensor_reduce(
                        ared[:],
                        ap(tmp, [[FCH, 128], [CO, PAIRS_PER_CHUNK], [16, C], [1, O]]),
                        axis=AX.X, op=ALU.add,
                    )
                    bsl = b_ij[:, k * PAIRS_PER_CHUNK * C:(k + 1) * PAIRS_PER_CHUNK * C]
                    nc.vector.tensor_tensor(bsl, bsl, ared[:], op=ALU.add)
                    # softmax over c
                    cexp = tmp_pool.tile([128, PAIRS_PER_CHUNK * C], f32, tag="ce")
                    nc.scalar.activation(cexp[:], bsl, ACTF.Exp)
                    csum = tmp_pool.tile([128, PAIRS_PER_CHUNK], f32, tag="cs")
                    nc.vector.tensor_reduce(
                        csum[:],
                        ap(cexp, [[PAIRS_PER_CHUNK * C, 128], [C, PAIRS_PER_CHUNK], [1, C]]),
                        axis=AX.X, op=ALU.add,
                    )
                    crec = tmp_pool.tile([128, PAIRS_PER_CHUNK], f32, tag="cr")
                    nc.vector.reciprocal(crec[:], csum[:])
                    cij = tmp_pool.tile([128, PAIRS_PER_CHUNK * C], f32, tag="cij")
                    nc.vector.tensor_tensor(
                        cij[:], cexp[:],
                        ap(crec, [[PAIRS_PER_CHUNK, 128], [1, PAIRS_PER_CHUNK], [0, C]]),
                        op=ALU.mult,
                    )
                    # s += sum_r cij * u
                    tmp2 = tmp_pool.tile([128, FCH], f32, tag="m2")
                    nc.vector.tensor_tensor(
                        tmp2[:], uch[:],
                        ap(cij, [[PAIRS_PER_CHUNK * C, 128], [C, PAIRS_PER_CHUNK], [1, C], [0, O]]),
                        op=ALU.mult,
                    )
                    sred = tmp_pool.tile([128, CO], f32, tag="sr")
                    nc.vector.tensor_reduce(
                        sred[:],
                        ap(tmp2, [[FCH, 128], [16, C], [1, O], [CO, PAIRS_PER_CHUNK]]),
                        axis=AX.X, op=ALU.add,
                    )
                    nc.vector.tensor_tensor(s_acc[:], s_acc[:], sred[:], op=ALU.add)
                squash_and_store(t)
    nc.compile()
    _cache["nc"] = nc
    return nc


def _host_prep(x, W):
    # x [B,R,I], W [1,R,C,O,I] -> xp [16, NP, 64] per core, wpair [16, NP, CO]
    Wr = np.ascontiguousarray(
        W[0].reshape(R, CO, I).transpose(2, 0, 1), dtype=np.float32
    )  # [I, R, CO]
    wpair = np.empty((16, NP, CO), np.float32)
    wpair[0:8] = Wr[:, 0::2, :]
    wpair[8:16] = Wr[:, 1::2, :]
    wps = np.ascontiguousarray(
        wpair.reshape(16, NP // 8, 8, CO).transpose(2, 0, 1, 3).reshape(128, NP // 8, CO)
    )
    maps = []
    for core in range(NCORES):
        xl = x[core * Bl:(core + 1) * Bl]          # [64, R, I]
        xp = np.empty((16, NP, 64), np.float32)
        xp[0:8] = xl[:, 0::2, :].transpose(2, 1, 0)
        xp[8:16] = xl[:, 1::2, :].transpose(2, 1, 0)
        xs = np.ascontiguousarray(
            xp.reshape(16, NP // 8, 8, 64).transpose(2, 0, 1, 3).reshape(128, NP // 8, 64)
        )
        maps.append({"xp": xp, "wpair": wpair, "xps": xs, "wps": wps})
    return maps


def kernel(x, W):
    x = np.asarray(x, dtype=np.float32)
    W = np.asarray(W, dtype=np.float32)
    nc = _build_program()
    in_maps = _host_prep(x, W)
    res = run_bass_kernel_spmd(nc, in_maps, list(range(NCORES))).results
    out = np.concatenate([r["v_out"] for r in res], axis=0)  # [B, CO]
    return out.reshape(B, C, O)



# revision 7
# speedup vs baseline: 110.9075x; 110.9075x over previous
import numpy as np
import ml_dtypes
import concourse.bass as bass
import concourse.bacc as bacc
import concourse.mybir as mybir
from concourse.bass_utils import run_bass_kernel_spmd
from concourse import tile

# DigitCapsules dynamic routing, data-parallel over batch on 8 cores.
# B=512, R=1152, C=10, O=16, I=8; per core Bl=64.
#
# v2: all-bf16 pipeline. Partitions p = parity*64 + b (r = 2*rp + parity).
# Free-dim layout is (o, c) [o outer stride C, c inner stride 1] so both
# broadcast multiplies (v over rp; c_ij over o) have step-1 innermost APs
# and hit the DVE 2x packed bf16 mode. u_hat chunks are regenerated per
# pass from bf16 canvas matmuls (FWL active), exited from PSUM by the
# scalar engine as bf16, and the two big products are split across
# gpsimd/vector; the o-reduce runs on gpsimd, the rp-reduce on vector.

NCORES = 8
B, R, C, O, I = 512, 1152, 10, 16, 8
Bl = B // NCORES          # 64 batch per core
CO = C * O                # 160 (free layout: o*C + c)
NP = R // 2               # 576 r-pairs
PAIRS_PER_CHUNK = 24      # 48 r per chunk; 8 psum banks x 3 pairs
NCHUNK = NP // PAIRS_PER_CHUNK  # 24
FCH = PAIRS_PER_CHUNK * CO      # 3840 free elems per chunk
GSPLIT = 7                # pairs of u*c_ij product on gpsimd; rest on vector
EPS = 1e-8

_cache = {}


def _build_program(nrep=1):
    key = ("nc", nrep)
    if key in _cache:
        return _cache[key]
    nc = bacc.Bacc("TRN2", target_bir_lowering=False, debug=False)
    f32 = mybir.dt.float32
    bf16 = mybir.dt.bfloat16
    xp_d = nc.dram_tensor("xp", [16, NP, 64], bf16, kind="ExternalInput")
    wpair_d = nc.dram_tensor("wpair", [16, NP, CO], bf16, kind="ExternalInput")
    NG = NP // 8              # 72 stacked groups of 8 pairs (K=128)
    xps_d = nc.dram_tensor("xps", [128, NG, 64], bf16, kind="ExternalInput")
    wps_d = nc.dram_tensor("wps", [128, NG, CO], bf16, kind="ExternalInput")
    out_d = nc.dram_tensor("v_out", [Bl, CO], f32, kind="ExternalOutput")

    AX = mybir.AxisListType
    ALU = mybir.AluOpType
    ACTF = mybir.ActivationFunctionType

    def ap(t, dims, offset=0):
        return bass.AP(t.tensor, offset, dims)

    with tile.TileContext(nc) as tc:
        with (
            tc.tile_pool(name="xpp", bufs=3) as xp_pool,
            tc.tile_pool(name="stk", bufs=2) as stk_pool,
            tc.tile_pool(name="cv", bufs=2) as cv_pool,
            tc.tile_pool(name="wp", bufs=2) as wp_pool,
            tc.tile_pool(name="psum", bufs=7, space="PSUM") as psum_pool,
            tc.tile_pool(name="ps1", bufs=1, space="PSUM") as ps1_pool,
            tc.tile_pool(name="uch", bufs=2) as uch_pool,
            tc.tile_pool(name="tmp", bufs=2) as tmp_pool,
            tc.tile_pool(name="res", bufs=1) as res_pool,
        ):
            b_ij = res_pool.tile([128, NP * C], f32, tag="bij")       # logits, (rp, c)
            s_acc = res_pool.tile([128, CO], f32, tag="sacc")
            vtile = res_pool.tile([128, CO], bf16, tag="vt")          # v on both halves
            s_fold = res_pool.tile([64, CO], f32, tag="sfold")
            sq = res_pool.tile([64, C], f32, tag="sq")
            f1 = res_pool.tile([64, C], f32, tag="f1")
            f2 = res_pool.tile([64, C], f32, tag="f2")

            def dma_xp_chunk(k):
                xpch = xp_pool.tile([16, PAIRS_PER_CHUNK * 64], bf16, tag="x")
                nc.sync.dma_start(
                    xpch[:],
                    xp_d[:, k * PAIRS_PER_CHUNK:(k + 1) * PAIRS_PER_CHUNK, :],
                )
                return xpch

            def squash_and_store(t):
                # fold parity halves, squash, broadcast v (or emit output)
                upper = tmp_pool.tile([64, CO], f32, tag="up")
                nc.sync.dma_start(upper[:], s_acc[64:128, :])
                nc.vector.tensor_tensor(s_fold[:], s_acc[0:64, :], upper[:], op=ALU.add)
                if t == 0:
                    nc.vector.tensor_scalar_mul(s_fold[:], s_fold[:], 0.1)
                prod = tmp_pool.tile([64, CO], f32, tag="pr")
                nc.vector.tensor_tensor(prod[:], s_fold[:], s_fold[:], op=ALU.mult)
                # sq[b, c] = sum_o prod[b, o, c]   (o stride C, c stride 1)
                nc.vector.tensor_reduce(
                    sq[:], ap(prod, [[CO, 64], [1, C], [C, O]]), axis=AX.X, op=ALU.add
                )
                onep = tmp_pool.tile([64, C], f32, tag="q1")
                nc.vector.tensor_scalar_add(onep[:], sq[:], 1.0)
                nc.vector.reciprocal(f1[:], onep[:])
                rt = tmp_pool.tile([64, C], f32, tag="q2")
                nc.vector.tensor_scalar_add(rt[:], sq[:], EPS)
                nc.scalar.activation(rt[:], rt[:], ACTF.Sqrt)
                nc.vector.reciprocal(f2[:], rt[:])
                nc.vector.tensor_tensor(f1[:], f1[:], f2[:], op=ALU.mult)
                nc.vector.tensor_tensor(f1[:], f1[:], sq[:], op=ALU.mult)
                # v = s * f1 broadcast over o (f1 per c)
                nc.vector.tensor_tensor(
                    vtile[0:64, :], s_fold[:], ap(f1, [[C, 64], [0, O], [1, C]]),
                    op=ALU.mult,
                )
                if t < 2:
                    nc.sync.dma_start(vtile[64:128, :], vtile[0:64, :])
                else:
                    vout = tmp_pool.tile([64, CO], f32, tag="vo")
                    nc.vector.tensor_tensor(
                        vout[:], s_fold[:], ap(f1, [[C, 64], [0, O], [1, C]]),
                        op=ALU.mult,
                    )
                    nc.sync.dma_start(out_d[:], vout[:])

            for _rep in range(nrep):
                nc.vector.memset(b_ij[:], 0.0)

                # ---- pass A: s_1 = sum_{r,i} x*W, PE only; K=128 stacked.
                s1ps = ps1_pool.tile([64, CO], f32, tag="s1")
                GA = 9  # stacked groups per chunk (72 total)
                for k in range(NG // GA):
                    xk = stk_pool.tile([128, GA * 64], bf16, tag="xk")
                    nc.sync.dma_start(xk[:], xps_d[:, k * GA:(k + 1) * GA, :])
                    wk = stk_pool.tile([128, GA * CO], bf16, tag="wk")
                    nc.sync.dma_start(wk[:], wps_d[:, k * GA:(k + 1) * GA, :])
                    for g in range(GA):
                        nc.tensor.matmul(
                            s1ps[:],
                            xk[:, g * 64:(g + 1) * 64],
                            wk[:, g * CO:(g + 1) * CO],
                            start=(k == 0 and g == 0),
                            stop=(k == NG // GA - 1 and g == GA - 1),
                        )
                nc.vector.memset(s_acc[:], 0.0)
                nc.vector.tensor_copy(s_acc[0:64, :], s1ps[:])
                squash_and_store(0)

                # ---- passes B (t=1), C (t=2): regen u_hat chunks + routing
                cv_count = 0
                for t in (1, 2):
                    nc.vector.memset(s_acc[:], 0.0)
                    for k in range(NCHUNK):
                        xpch = dma_xp_chunk(k)
                        wch = wp_pool.tile([16, PAIRS_PER_CHUNK * CO], bf16, tag="w")
                        nc.sync.dma_start(
                            wch[:],
                            wpair_d[:, k * PAIRS_PER_CHUNK:(k + 1) * PAIRS_PER_CHUNK, :],
                        )
                        # stage block-diagonal canvases: [16, 24*128]
                        cch = cv_pool.tile([16, PAIRS_PER_CHUNK * 128], bf16, tag="c")
                        if cv_count < 2:
                            nc.vector.memset(cch[:], 0.0)
                        cv_count += 1
                        cpitch = PAIRS_PER_CHUNK * 128
                        xpitch = PAIRS_PER_CHUNK * 64
                        nc.sync.dma_start(
                            ap(cch, [[cpitch, 8], [128, PAIRS_PER_CHUNK], [1, 64]], 0),
                            ap(xpch, [[xpitch, 8], [64, PAIRS_PER_CHUNK], [1, 64]], 0),
                        )
                        nc.sync.dma_start(
                            ap(cch, [[cpitch, 8], [128, PAIRS_PER_CHUNK], [1, 64]],
                               8 * cpitch + 64),
                            ap(xpch, [[xpitch, 8], [64, PAIRS_PER_CHUNK], [1, 64]],
                               8 * xpitch),
                        )
                        uch = uch_pool.tile([128, FCH], bf16, tag="u")
                        for j in range(8):  # 8 psum tiles, 3 pairs each
                            ps = psum_pool.tile([128, 3 * CO], f32, tag="ps")
                            for q in range(3):
                                rp = j * 3 + q
                                nc.tensor.matmul(
                                    ps[:, q * CO:(q + 1) * CO],
                                    cch[:, rp * 128:(rp + 1) * 128],
                                    wch[:, rp * CO:(rp + 1) * CO],
                                    start=True, stop=True,
                                )
                            nc.scalar.copy(
                                uch[:, j * 3 * CO:(j + 1) * 3 * CO], ps[:]
                            )

                        # uch free dims: (rp 24, o 16, c 10); strides rp=160, o=10, c=1
                        # tmp = u * v (v broadcast over rp); all on gpsimd to
                        # offload the vector engine (the wall in this design)
                        tmp = tmp_pool.tile([128, FCH], bf16, tag="m1")
                        nc.gpsimd.tensor_tensor(
                            tmp[:], uch[:],
                            ap(vtile, [[CO, 128], [0, PAIRS_PER_CHUNK], [1, CO]]),
                            op=ALU.mult,
                        )
                        # a[p,(rp,c)] = sum_o tmp  (reduce over o, stride C)
                        ared = tmp_pool.tile([128, PAIRS_PER_CHUNK * C], f32, tag="ar")
                        nc.vector.tensor_reduce(
                            ared[:],
                            ap(tmp, [[FCH, 128], [CO, PAIRS_PER_CHUNK], [1, C], [C, O]]),
                            axis=AX.X, op=ALU.add,
                        )
                        bsl = b_ij[:, k * PAIRS_PER_CHUNK * C:(k + 1) * PAIRS_PER_CHUNK * C]
                        nc.vector.tensor_tensor(bsl, bsl, ared[:], op=ALU.add)
                        # softmax over c
                        cexp = tmp_pool.tile([128, PAIRS_PER_CHUNK * C], bf16, tag="ce")
                        nc.scalar.activation(cexp[:], bsl, ACTF.Exp)
                        csum = tmp_pool.tile([128, PAIRS_PER_CHUNK], f32, tag="cs")
                        nc.vector.tensor_reduce(
                            csum[:],
                            ap(cexp, [[PAIRS_PER_CHUNK * C, 128], [C, PAIRS_PER_CHUNK], [1, C]]),
                            axis=AX.X, op=ALU.add,
                        )
                        crec = tmp_pool.tile([128, PAIRS_PER_CHUNK], f32, tag="cr")
                        nc.vector.reciprocal(crec[:], csum[:])
                        cij = tmp_pool.tile([128, PAIRS_PER_CHUNK * C], bf16, tag="cij")
                        nc.vector.tensor_tensor(
                            cij[:], cexp[:],
                            ap(crec, [[PAIRS_PER_CHUNK, 128], [1, PAIRS_PER_CHUNK], [0, C]]),
                            op=ALU.mult,
                        )
                        # tmp2 = u * c_ij (c_ij broadcast over o); gpsimd takes
                        # GSPLIT pairs, vector the rest (engine balancing)
                        tmp2 = tmp_pool.tile([128, FCH], bf16, tag="m2")
                        nc.gpsimd.tensor_tensor(
                            tmp2[:, :GSPLIT * CO], uch[:, :GSPLIT * CO],
                            ap(cij, [[PAIRS_PER_CHUNK * C, 128], [C, GSPLIT], [0, O], [1, C]]),
                            op=ALU.mult,
                        )
                        nc.vector.tensor_tensor(
                            tmp2[:, GSPLIT * CO:], uch[:, GSPLIT * CO:],
                            ap(cij, [[PAIRS_PER_CHUNK * C, 128], [C, PAIRS_PER_CHUNK - GSPLIT], [0, O], [1, C]],
                               GSPLIT * C),
                            op=ALU.mult,
                        )
                        # s partial: reduce over rp (stride CO)
                        sred = tmp_pool.tile([128, CO], f32, tag="sr")
                        nc.vector.tensor_reduce(
                            sred[:],
                            ap(tmp2, [[FCH, 128], [C, O], [1, C], [CO, PAIRS_PER_CHUNK]]),
                            axis=AX.X, op=ALU.add,
                        )
                        nc.vector.tensor_tensor(s_acc[:], s_acc[:], sred[:], op=ALU.add)
                    squash_and_store(t)
    nc.compile()
    _cache[key] = nc
    return nc


def _host_prep(x, W):
    # x [B,R,I], W [1,R,C,O,I] -> xp [16, NP, 64] per core, wpair [16, NP, CO]
    # free layout inside CO is (o, c): index = o*C + c
    bf = ml_dtypes.bfloat16
    Wr = np.ascontiguousarray(
        W[0].transpose(0, 2, 1, 3).reshape(R, CO, I).transpose(2, 0, 1)
    ).astype(bf)  # [I, R, CO] with CO = (o, c)
    wpair = np.empty((16, NP, CO), bf)
    wpair[0:8] = Wr[:, 0::2, :]
    wpair[8:16] = Wr[:, 1::2, :]
    wps = np.ascontiguousarray(
        wpair.reshape(16, NP // 8, 8, CO).transpose(2, 0, 1, 3).reshape(128, NP // 8, CO)
    )
    maps = []
    for core in range(NCORES):
        xl = x[core * Bl:(core + 1) * Bl]          # [64, R, I]
        xp = np.empty((16, NP, 64), bf)
        xp[0:8] = xl[:, 0::2, :].transpose(2, 1, 0).astype(bf)
        xp[8:16] = xl[:, 1::2, :].transpose(2, 1, 0).astype(bf)
        xs = np.ascontiguousarray(
            xp.reshape(16, NP // 8, 8, 64).transpose(2, 0, 1, 3).reshape(128, NP // 8, 64)
        )
        maps.append({"xp": xp, "wpair": wpair, "xps": xs, "wps": wps})
    return maps


def kernel(x, W):
    x = np.asarray(x, dtype=np.float32)
    W = np.asarray(W, dtype=np.float32)
    nc = _build_program()
    in_maps = _host_prep(x, W)
    res = run_bass_kernel_spmd(nc, in_maps, list(range(NCORES))).results
    out = np.concatenate([r["v_out"] for r in res], axis=0)  # [B, CO] (o, c)
    return np.ascontiguousarray(
        out.reshape(B, O, C).transpose(0, 2, 1)
    )  # [B, C, O]


# revision 10
# speedup vs baseline: 136.8891x; 1.2343x over previous
import numpy as np
import ml_dtypes
import concourse.bass as bass
import concourse.bacc as bacc
import concourse.mybir as mybir
from concourse.bass_utils import run_bass_kernel_spmd
from concourse import tile

# DigitCapsules dynamic routing, data-parallel over batch on 8 cores.
# B=512, R=1152, C=10, O=16, I=8; per core Bl=64.
#
# v2: all-bf16 pipeline. Partitions p = parity*64 + b (r = 2*rp + parity).
# Free-dim layout is (o, c) [o outer stride C, c inner stride 1] so both
# broadcast multiplies (v over rp; c_ij over o) have step-1 innermost APs
# and hit the DVE 2x packed bf16 mode. u_hat chunks are regenerated per
# pass from bf16 canvas matmuls (FWL active), exited from PSUM by the
# scalar engine as bf16, and the two big products are split across
# gpsimd/vector; the o-reduce runs on gpsimd, the rp-reduce on vector.

NCORES = 8
B, R, C, O, I = 512, 1152, 10, 16, 8
Bl = B // NCORES          # 64 batch per core
CO = C * O                # 160 (free layout: o*C + c)
NP = R // 2               # 576 r-pairs
PAIRS_PER_CHUNK = 24      # 48 r per chunk; 8 psum banks x 3 pairs
NCHUNK = NP // PAIRS_PER_CHUNK  # 24
FCH = PAIRS_PER_CHUNK * CO      # 3840 free elems per chunk
TMPG = 20                 # pairs of u*v product on gpsimd; rest on vector
EPS = 1e-8

_cache = {}


def _build_program(nrep=1):
    key = ("nc", nrep)
    if key in _cache:
        return _cache[key]
    nc = bacc.Bacc("TRN2", target_bir_lowering=False, debug=False)
    f32 = mybir.dt.float32
    bf16 = mybir.dt.bfloat16
    xp_d = nc.dram_tensor("xp", [16, NP, 64], bf16, kind="ExternalInput")
    wpair_d = nc.dram_tensor("wpair", [16, NP, CO], bf16, kind="ExternalInput")
    NG = NP // 8              # 72 stacked groups of 8 pairs (K=128)
    xps_d = nc.dram_tensor("xps", [128, NG, 64], bf16, kind="ExternalInput")
    wps_d = nc.dram_tensor("wps", [128, NG, CO], bf16, kind="ExternalInput")
    out_d = nc.dram_tensor("v_out", [Bl, CO], f32, kind="ExternalOutput")

    AX = mybir.AxisListType
    ALU = mybir.AluOpType
    ACTF = mybir.ActivationFunctionType

    def ap(t, dims, offset=0):
        return bass.AP(t.tensor, offset, dims)

    with tile.TileContext(nc) as tc:
        with (
            tc.tile_pool(name="xpp", bufs=3) as xp_pool,
            tc.tile_pool(name="stk", bufs=2) as stk_pool,
            tc.tile_pool(name="cv", bufs=2) as cv_pool,
            tc.tile_pool(name="wp", bufs=2) as wp_pool,
            tc.tile_pool(name="psum", bufs=7, space="PSUM") as psum_pool,
            tc.tile_pool(name="ps1", bufs=1, space="PSUM") as ps1_pool,
            tc.tile_pool(name="uch", bufs=2) as uch_pool,
            tc.tile_pool(name="tmp", bufs=2) as tmp_pool,
            tc.tile_pool(name="res", bufs=1) as res_pool,
        ):
            b_ij = res_pool.tile([128, NP * C], f32, tag="bij")       # logits, (rp, c)
            s_acc = res_pool.tile([128, CO], f32, tag="sacc")
            vtile = res_pool.tile([128, CO], bf16, tag="vt")          # v on both halves
            s_fold = res_pool.tile([64, CO], f32, tag="sfold")
            sq = res_pool.tile([64, C], f32, tag="sq")
            f1 = res_pool.tile([64, C], f32, tag="f1")
            f2 = res_pool.tile([64, C], f32, tag="f2")

            def dma_xp_chunk(k):
                xpch = xp_pool.tile([16, PAIRS_PER_CHUNK * 64], bf16, tag="x")
                nc.sync.dma_start(
                    xpch[:],
                    xp_d[:, k * PAIRS_PER_CHUNK:(k + 1) * PAIRS_PER_CHUNK, :],
                )
                return xpch

            def squash_and_store(t):
                # fold parity halves, squash, broadcast v (or emit output)
                upper = tmp_pool.tile([64, CO], f32, tag="up")
                nc.sync.dma_start(upper[:], s_acc[64:128, :])
                nc.vector.tensor_tensor(s_fold[:], s_acc[0:64, :], upper[:], op=ALU.add)
                if t == 0:
                    nc.vector.tensor_scalar_mul(s_fold[:], s_fold[:], 0.1)
                prod = tmp_pool.tile([64, CO], f32, tag="pr")
                nc.vector.tensor_tensor(prod[:], s_fold[:], s_fold[:], op=ALU.mult)
                # sq[b, c] = sum_o prod[b, o, c]   (o stride C, c stride 1)
                nc.vector.tensor_reduce(
                    sq[:], ap(prod, [[CO, 64], [1, C], [C, O]]), axis=AX.X, op=ALU.add
                )
                onep = tmp_pool.tile([64, C], f32, tag="q1")
                nc.vector.tensor_scalar_add(onep[:], sq[:], 1.0)
                nc.vector.reciprocal(f1[:], onep[:])
                rt = tmp_pool.tile([64, C], f32, tag="q2")
                nc.vector.tensor_scalar_add(rt[:], sq[:], EPS)
                nc.scalar.activation(rt[:], rt[:], ACTF.Sqrt)
                nc.vector.reciprocal(f2[:], rt[:])
                nc.vector.tensor_tensor(f1[:], f1[:], f2[:], op=ALU.mult)
                nc.vector.tensor_tensor(f1[:], f1[:], sq[:], op=ALU.mult)
                # v = s * f1 broadcast over o (f1 per c)
                nc.vector.tensor_tensor(
                    vtile[0:64, :], s_fold[:], ap(f1, [[C, 64], [0, O], [1, C]]),
                    op=ALU.mult,
                )
                if t < 2:
                    nc.sync.dma_start(vtile[64:128, :], vtile[0:64, :])
                else:
                    vout = tmp_pool.tile([64, CO], f32, tag="vo")
                    nc.vector.tensor_tensor(
                        vout[:], s_fold[:], ap(f1, [[C, 64], [0, O], [1, C]]),
                        op=ALU.mult,
                    )
                    nc.sync.dma_start(out_d[:], vout[:])

            for _rep in range(nrep):
                nc.vector.memset(b_ij[:], 0.0)

                # ---- pass A: s_1 = sum_{r,i} x*W, PE only; K=128 stacked.
                s1ps = ps1_pool.tile([64, CO], f32, tag="s1")
                GA = 9  # stacked groups per chunk (72 total)
                for k in range(NG // GA):
                    xk = stk_pool.tile([128, GA * 64], bf16, tag="xk")
                    nc.sync.dma_start(xk[:], xps_d[:, k * GA:(k + 1) * GA, :])
                    wk = stk_pool.tile([128, GA * CO], bf16, tag="wk")
                    nc.sync.dma_start(wk[:], wps_d[:, k * GA:(k + 1) * GA, :])
                    for g in range(GA):
                        nc.tensor.matmul(
                            s1ps[:],
                            xk[:, g * 64:(g + 1) * 64],
                            wk[:, g * CO:(g + 1) * CO],
                            start=(k == 0 and g == 0),
                            stop=(k == NG // GA - 1 and g == GA - 1),
                        )
                nc.vector.memset(s_acc[:], 0.0)
                nc.vector.tensor_copy(s_acc[0:64, :], s1ps[:])
                squash_and_store(0)

                # ---- passes B (t=1), C (t=2): regen u_hat chunks + routing.
                # Two-stage software pipeline: stage1(k) = u_hat gen + u*v +
                # a-reduce + logit update + exp; stage2(k) = softmax tail +
                # u*c + s-reduce, issued one chunk behind so no engine FIFO
                # head-of-line blocks on just-produced data.
                cv_count = 0
                for t in (1, 2):
                    nc.vector.memset(s_acc[:], 0.0)
                    live = {}

                    def stage1(k):
                        nonlocal cv_count
                        xpch = dma_xp_chunk(k)
                        wch = wp_pool.tile([16, PAIRS_PER_CHUNK * CO], bf16, tag="w")
                        nc.sync.dma_start(
                            wch[:],
                            wpair_d[:, k * PAIRS_PER_CHUNK:(k + 1) * PAIRS_PER_CHUNK, :],
                        )
                        # stage block-diagonal canvases: [16, 24*128]
                        cch = cv_pool.tile([16, PAIRS_PER_CHUNK * 128], bf16, tag="c")
                        if cv_count < 2:
                            nc.vector.memset(cch[:], 0.0)
                        cv_count += 1
                        cpitch = PAIRS_PER_CHUNK * 128
                        xpitch = PAIRS_PER_CHUNK * 64
                        nc.sync.dma_start(
                            ap(cch, [[cpitch, 8], [128, PAIRS_PER_CHUNK], [1, 64]], 0),
                            ap(xpch, [[xpitch, 8], [64, PAIRS_PER_CHUNK], [1, 64]], 0),
                        )
                        nc.sync.dma_start(
                            ap(cch, [[cpitch, 8], [128, PAIRS_PER_CHUNK], [1, 64]],
                               8 * cpitch + 64),
                            ap(xpch, [[xpitch, 8], [64, PAIRS_PER_CHUNK], [1, 64]],
                               8 * xpitch),
                        )
                        uch = uch_pool.tile([128, FCH], bf16, tag="u")
                        for j in range(8):  # 8 psum tiles, 3 pairs each
                            ps = psum_pool.tile([128, 3 * CO], f32, tag="ps")
                            for q in range(3):
                                rp = j * 3 + q
                                nc.tensor.matmul(
                                    ps[:, q * CO:(q + 1) * CO],
                                    cch[:, rp * 128:(rp + 1) * 128],
                                    wch[:, rp * CO:(rp + 1) * CO],
                                    start=True, stop=True,
                                )
                            nc.scalar.copy(
                                uch[:, j * 3 * CO:(j + 1) * 3 * CO], ps[:]
                            )

                        # uch free dims: (rp 24, o 16, c 10); strides rp=160, o=10, c=1
                        # tmp = u * v (v broadcast over rp): gpsimd, with a
                        # small vector share for balance
                        tmp = tmp_pool.tile([128, FCH], bf16, tag="m1")
                        nc.gpsimd.tensor_tensor(
                            tmp[:, :TMPG * CO], uch[:, :TMPG * CO],
                            ap(vtile, [[CO, 128], [0, TMPG], [1, CO]]),
                            op=ALU.mult,
                        )
                        nc.vector.tensor_tensor(
                            tmp[:, TMPG * CO:], uch[:, TMPG * CO:],
                            ap(vtile, [[CO, 128], [0, PAIRS_PER_CHUNK - TMPG], [1, CO]]),
                            op=ALU.mult,
                        )
                        # a[p,(rp,c)] = sum_o tmp: in-place halving tree over o
                        # (dense step-1 runs keep the 2x packed mode)
                        for w in (80, 40, 20):
                            nc.vector.tensor_tensor(
                                ap(tmp, [[FCH, 128], [CO, PAIRS_PER_CHUNK], [1, w]]),
                                ap(tmp, [[FCH, 128], [CO, PAIRS_PER_CHUNK], [1, w]]),
                                ap(tmp, [[FCH, 128], [CO, PAIRS_PER_CHUNK], [1, w]], w),
                                op=ALU.add,
                            )
                        ared = tmp_pool.tile([128, PAIRS_PER_CHUNK * C], f32, tag="ar")
                        nc.vector.tensor_tensor(
                            ap(ared, [[PAIRS_PER_CHUNK * C, 128], [C, PAIRS_PER_CHUNK], [1, C]]),
                            ap(tmp, [[FCH, 128], [CO, PAIRS_PER_CHUNK], [1, C]]),
                            ap(tmp, [[FCH, 128], [CO, PAIRS_PER_CHUNK], [1, C]], C),
                            op=ALU.add,
                        )
                        bsl = b_ij[:, k * PAIRS_PER_CHUNK * C:(k + 1) * PAIRS_PER_CHUNK * C]
                        nc.vector.tensor_tensor(bsl, bsl, ared[:], op=ALU.add)
                        cexp = tmp_pool.tile([128, PAIRS_PER_CHUNK * C], bf16, tag="ce")
                        nc.scalar.activation(cexp[:], bsl, ACTF.Exp)
                        live[k] = (uch, cexp)

                    def stage2(k):
                        uch, cexp = live.pop(k)
                        csum = tmp_pool.tile([128, PAIRS_PER_CHUNK], f32, tag="cs")
                        nc.vector.tensor_reduce(
                            csum[:],
                            ap(cexp, [[PAIRS_PER_CHUNK * C, 128], [C, PAIRS_PER_CHUNK], [1, C]]),
                            axis=AX.X, op=ALU.add,
                        )
                        crec = tmp_pool.tile([128, PAIRS_PER_CHUNK], f32, tag="cr")
                        nc.vector.reciprocal(crec[:], csum[:])
                        cij = tmp_pool.tile([128, PAIRS_PER_CHUNK * C], bf16, tag="cij")
                        nc.vector.tensor_tensor(
                            cij[:], cexp[:],
                            ap(crec, [[PAIRS_PER_CHUNK, 128], [1, PAIRS_PER_CHUNK], [0, C]]),
                            op=ALU.mult,
                        )
                        # tmp2 = u * c_ij (c_ij broadcast over o)
                        tmp2 = tmp_pool.tile([128, FCH], bf16, tag="m2")
                        nc.vector.tensor_tensor(
                            tmp2[:], uch[:],
                            ap(cij, [[PAIRS_PER_CHUNK * C, 128], [C, PAIRS_PER_CHUNK], [0, O], [1, C]]),
                            op=ALU.mult,
                        )
                        # s partial: in-place halving tree over rp (dense slices)
                        for w in (1920, 960, 480):
                            nc.vector.tensor_tensor(
                                tmp2[:, :w], tmp2[:, :w], tmp2[:, w:2 * w], op=ALU.add,
                            )
                        nc.vector.tensor_tensor(
                            tmp2[:, :CO], tmp2[:, :CO], tmp2[:, 2 * CO:3 * CO], op=ALU.add,
                        )
                        nc.vector.tensor_tensor(s_acc[:], s_acc[:], tmp2[:, :CO], op=ALU.add)
                        nc.vector.tensor_tensor(s_acc[:], s_acc[:], tmp2[:, CO:2 * CO], op=ALU.add)

                    for k in range(NCHUNK):
                        # stage2(k-1) first: its vector work sits AHEAD of
                        # stage1(k)'s in the strict-FIFO vector queue, so the
                        # vector engine never head-of-line blocks on gpsimd's
                        # tmp_k product.
                        if k > 0:
                            stage2(k - 1)
                        stage1(k)
                    stage2(NCHUNK - 1)
                    squash_and_store(t)
    nc.compile()
    _cache[key] = nc
    return nc


def _host_prep(x, W):
    # x [B,R,I], W [1,R,C,O,I] -> xp [16, NP, 64] per core, wpair [16, NP, CO]
    # free layout inside CO is (o, c): index = o*C + c
    bf = ml_dtypes.bfloat16
    Wr = np.ascontiguousarray(
        W[0].transpose(0, 2, 1, 3).reshape(R, CO, I).transpose(2, 0, 1)
    ).astype(bf)  # [I, R, CO] with CO = (o, c)
    wpair = np.empty((16, NP, CO), bf)
    wpair[0:8] = Wr[:, 0::2, :]
    wpair[8:16] = Wr[:, 1::2, :]
    wps = np.ascontiguousarray(
        wpair.reshape(16, NP // 8, 8, CO).transpose(2, 0, 1, 3).reshape(128, NP // 8, CO)
    )
    maps = []
    for core in range(NCORES):
        xl = x[core * Bl:(core + 1) * Bl]          # [64, R, I]
        xp = np.empty((16, NP, 64), bf)
        xp[0:8] = xl[:, 0::2, :].transpose(2, 1, 0).astype(bf)
        xp[8:16] = xl[:, 1::2, :].transpose(2, 1, 0).astype(bf)
        xs = np.ascontiguousarray(
            xp.reshape(16, NP // 8, 8, 64).transpose(2, 0, 1, 3).reshape(128, NP // 8, 64)
        )
        maps.append({"xp": xp, "wpair": wpair, "xps": xs, "wps": wps})
    return maps


def kernel(x, W):
    x = np.asarray(x, dtype=np.float32)
    W = np.asarray(W, dtype=np.float32)
    nc = _build_program()
    in_maps = _host_prep(x, W)
    res = run_bass_kernel_spmd(nc, in_maps, list(range(NCORES))).results
    out = np.concatenate([r["v_out"] for r in res], axis=0)  # [B, CO] (o, c)
    return np.ascontiguousarray(
        out.reshape(B, O, C).transpose(0, 2, 1)
    )  # [B, C, O]
